# revision 37
# baseline (speedup 1.0000x reference)
"""ArcNegFace loss kernel for 8 TRN2 NeuronCores.

Strategy: model-parallel classification head. The weight matrix
[100000, 512] is sharded over its out_features axis across the 8 cores
(padded to 102400 rows -> 12800 rows / core, processed as 12 column
tiles of 1024 plus one of 512). Each core computes its [256, 12800]
slice of the logits.

The label-gather is done host-side (gather of 256 weight rows,
replicated to every core); each core recomputes cos_lb / a_lb in f32
locally (tiny), so no collective is needed. The one-hot "positive"
logits (256 scalars) are patched host-side from a device-computed a_lb
output during unsharding.

Per-core dataflow (software-pipelined by Tile across column tiles):
  HBM --SWDGE cast f32->fp16--> w_nat [128c, nj, 512d]
  ssq_c   = sum_d w^2         (Square+accum_out on ACT / STT+accum on
                               DVE, split by SSQ_DVE_OF_20 for balance)
  rnorm   = rsqrt(ssq)        (DVE-only: quake bit-trick seed + 2
                               Newton steps; avoids ACT Sqrt and its
                               activation-table thrash)
  wn      = w * rnorm         (per-partition tensor_scalar, fp16)
  wT      = one merged xbar DMA-transpose per tile (fp16 SBUF->SBUF,
                               [128, nj*512] -> [128, nj*4, 128])
  cos     = xnT.T @ wT        (PE, K=512 accumulated in PSUM, fp16)
  d2      = Square(cos - a)   (ACT, PSUM src, per-partition bias)
  f       = Exp(-d2/sigma + ln(SCALE*ALPHA))   (ACT, K1 folded in bias)
  s       = (cos + 1) * f     (DVE scalar_tensor_tensor, PSUM src)
  o       = s - SCALE         (DVE tensor_scalar, fp16 out)
  HBM <-- o (fp16; host casts to f32)
"""

import math

import numpy as np

B, D, C = 256, 512, 100000
NCORES = 8
CSH = 12800                 # padded columns per core
CPAD = CSH * NCORES         # 102400
# column tiles per core: two 512s to ramp the pipeline fast, then 1024s
CT_SIZES = [512, 512] + [1024] * 11 + [512]
# Newton-rsqrt batches: groups of tiles solved together
CT_GROUPS = [[0], [1], [2, 3], [4, 5, 6], [7, 8, 9], [10, 11, 12], [13]]
SCALE = 64.0
MARGIN = 0.5
ALPHA = 1.2
SIGMA = 2.0
THRESH = math.cos(math.pi - MARGIN)
MM_ = math.sin(math.pi - MARGIN) * MARGIN
COS_M = math.cos(MARGIN)
SIN_M = math.sin(MARGIN)
K1 = SCALE * ALPHA
LNK1 = math.log(K1)

# Of every 20 ssq column-slices, this many run on DVE (STT + accum_out);
# the rest run on ACT (Square + accum_out). Balances the two engines.
SSQ_DVE_OF_20 = 8

_CACHE: dict = {}


def _build():
    from contextlib import ExitStack

    import concourse.bacc as bacc
    import concourse.bass as bass
    import concourse.tile as tile
    from concourse import mybir

    f32 = mybir.dt.float32
    f16 = mybir.dt.float16
    Alu = mybir.AluOpType
    Act = mybir.ActivationFunctionType

    nc = bacc.Bacc(
        "TRN2", target_bir_lowering=False, debug=False, num_devices=NCORES
    )
    inp_e = nc.dram_tensor("inp", [B, D], f32, kind="ExternalInput").ap()
    wlab_e = nc.dram_tensor("wlab", [B, D], f32, kind="ExternalInput").ap()
    w_e = nc.dram_tensor("w", [CSH, D], f32, kind="ExternalInput").ap()
    out_e = nc.dram_tensor("out", [B, CSH], f16, kind="ExternalOutput").ap()
    alb_e = nc.dram_tensor("alb", [128, 2], f32, kind="ExternalOutput").ap()

    with tile.TileContext(nc) as tc, ExitStack() as ctx:
        singles = ctx.enter_context(tc.tile_pool(name="singles", bufs=1))
        wpool = ctx.enter_context(tc.tile_pool(name="wpool", bufs=5))
        wtpool = ctx.enter_context(tc.tile_pool(name="wtpool", bufs=3))
        spool = ctx.enter_context(tc.tile_pool(name="spool", bufs=4))
        tpool = ctx.enter_context(tc.tile_pool(name="tpool", bufs=3))
        epool = ctx.enter_context(tc.tile_pool(name="epool", bufs=3))
        opool = ctx.enter_context(tc.tile_pool(name="opool", bufs=3))
        psum = ctx.enter_context(tc.tile_pool(name="psum", bufs=4, space="PSUM"))

        # int32 constants for the Newton-rsqrt bit-trick seed
        c_shift = singles.tile([128, 1], mybir.dt.int32)
        nc.vector.memset(c_shift, 1)
        c_xor = singles.tile([128, 1], mybir.dt.int32)
        nc.vector.memset(c_xor, -1)
        c_magic = singles.tile([128, 1], mybir.dt.int32)
        nc.vector.memset(c_magic, 0x5F3759E0)   # 0x5f3759df + 1

        def rsqrt_dve(pool, src_ap, w, name, iters=2):
            """rsqrt via quake bit-trick seed + Newton (DVE only; keeps
            the ScalarE activation-table set untouched)."""
            hh = pool.tile([128, w], mybir.dt.int32, name=f"{name}_h")
            iv = src_ap.bitcast(mybir.dt.int32)
            bs = (128, w)
            nc.vector.tensor_tensor(hh, iv, c_shift.to_broadcast(bs),
                                    Alu.arith_shift_right)
            nc.vector.tensor_tensor(hh, hh, c_xor.to_broadcast(bs),
                                    Alu.bitwise_xor)
            nc.vector.tensor_tensor(hh, hh, c_magic.to_broadcast(bs),
                                    Alu.add)
            yv = hh.bitcast(f32)
            nt = pool.tile([128, w], f32, name=f"{name}_n")
            for _ in range(iters):
                nc.vector.tensor_tensor(nt, yv, yv, Alu.mult)
                nc.vector.tensor_tensor(nt, nt, src_ap, Alu.mult)
                nc.vector.tensor_scalar(nt, nt, -0.5, 1.5,
                                        Alu.mult, Alu.add)
                nc.vector.tensor_tensor(yv, yv, nt, Alu.mult)
            return yv

        # ---------------- x / wlab prep (tiny, one-time) ----------------
        xt = singles.tile([128, 2, D], f32)
        nc.gpsimd.dma_start(xt, inp_e.rearrange("(j p) d -> p j d", p=128))
        wl = singles.tile([128, 2, D], f32)
        nc.gpsimd.dma_start(wl, wlab_e.rearrange("(j p) d -> p j d", p=128))

        ssqx = singles.tile([128, 2], f32)
        ssql = singles.tile([128, 2], f32)
        for j in range(2):
            tr = tpool.tile([128, D], f32, tag="preptrash")
            nc.scalar.activation(tr, xt[:, j], Act.Square,
                                 accum_out=ssqx[:, j:j + 1])
            tr = tpool.tile([128, D], f32, tag="preptrash")
            nc.scalar.activation(tr, wl[:, j], Act.Square,
                                 accum_out=ssql[:, j:j + 1])

        rnx = rsqrt_dve(singles, ssqx, 2, "rnx", iters=3)
        rnl = rsqrt_dve(singles, ssql, 2, "rnl", iters=3)

        xn16 = singles.tile([128, 2, D], f16)
        xnf = singles.tile([128, 2, D], f32)
        wlf = singles.tile([128, 2, D], f32)
        for j in range(2):
            nc.vector.tensor_scalar(xn16[:, j], xt[:, j], rnx[:, j:j + 1],
                                    None, Alu.mult)
            nc.vector.tensor_scalar(xnf[:, j], xt[:, j], rnx[:, j:j + 1],
                                    None, Alu.mult)
            nc.vector.tensor_scalar(wlf[:, j], wl[:, j], rnl[:, j:j + 1],
                                    None, Alu.mult)

        # cos_lb[b] = xn[b] . wn_label[b]   (f32)
        coslb = singles.tile([128, 2], f32)
        for j in range(2):
            tr = tpool.tile([128, D], f32, tag="preptrash")
            nc.vector.scalar_tensor_tensor(
                tr, xnf[:, j], 1.0, wlf[:, j], Alu.mult, Alu.mult,
                accum_out=coslb[:, j:j + 1])

        # a_lb = cos_lb > THRESH ? cos(acos(clip(cos_lb)) + m) : cos_lb - mm
        #      = c*cos(m) - sin(m)*sqrt(1-c^2)   (branch 1, c clipped)
        cmin = singles.tile([128, 2], f32)
        nc.vector.tensor_scalar(cmin, coslb, 1.0, -1.0, Alu.min, Alu.max)
        csq = singles.tile([128, 2], f32)
        nc.scalar.activation(csq, cmin, Act.Square)
        y1 = singles.tile([128, 2], f32)
        nc.vector.tensor_scalar(y1, csq, -1.0, 1.0, Alu.mult, Alu.add)
        nc.vector.tensor_scalar(y1, y1, 1e-20, None, Alu.max)
        # sqrt(y1) = y1 * rsqrt(y1)
        ry1 = rsqrt_dve(singles, y1, 2, "ry1", iters=3)
        sn = singles.tile([128, 2], f32)
        nc.vector.tensor_tensor(sn, y1, ry1, Alu.mult)
        b1 = singles.tile([128, 2], f32)
        nc.vector.tensor_scalar(b1, cmin, COS_M, None, Alu.mult)
        snm = singles.tile([128, 2], f32)
        nc.vector.tensor_scalar(snm, sn, -SIN_M, None, Alu.mult)
        nc.vector.tensor_tensor(b1, b1, snm, Alu.add)
        b2 = singles.tile([128, 2], f32)
        nc.vector.tensor_scalar(b2, coslb, MM_, None, Alu.subtract)
        mask = singles.tile([128, 2], mybir.dt.uint8)
        nc.vector.tensor_scalar(mask, coslb, THRESH, None, Alu.is_gt)
        alb = singles.tile([128, 2], f32)
        nc.vector.select(alb, mask, b1, b2)
        nega = singles.tile([128, 2], f32)
        nc.vector.tensor_scalar(nega, alb, -1.0, None, Alu.mult)
        nc.gpsimd.dma_start(alb_e, alb)

        lnk1 = singles.tile([128, 1], f32)
        nc.vector.memset(lnk1, LNK1)

        # xnT[p, j2, k, b] = xn[j2*128 + b, k*128 + p]   (fp16)
        xnT = singles.tile([128, 2, 4, 128], f16)
        nc.sync.dma_start_transpose(xnT, xn16)

        # ---------------- main loop over column-tile groups -------------
        ct_start = [0]
        for c in CT_SIZES:
            ct_start.append(ct_start[-1] + c)
        idx = 0
        for grp in CT_GROUPS:
            njs = [CT_SIZES[t] // 128 for t in grp]     # j-groups per tile
            tot_j = sum(njs)
            # ssq for all tiles of the group -> one Newton rsqrt solve
            ssqg = spool.tile([128, tot_j], f32, tag="ssqg",
                              name=f"ssqg{grp[0]}")
            wnats = []
            joff = 0
            for gi, t in enumerate(grp):
                nj = njs[gi]
                cols = CT_SIZES[t]
                wnat = wpool.tile([128, nj, D], f16, tag="wnat",
                                  name=f"wnat{t}")
                nc.gpsimd.dma_start(
                    wnat,
                    w_e[ct_start[t]:ct_start[t] + cols].rearrange(
                        "(j p) d -> p j d", p=128))
                wnats.append(wnat)
                for j in range(nj):
                    acc = ssqg[:, joff + j:joff + j + 1]
                    if (idx % 20) < SSQ_DVE_OF_20:
                        tr16 = tpool.tile([128, D], f16, tag="trash16",
                                          name=f"trd{t}_{j}")
                        nc.vector.scalar_tensor_tensor(
                            tr16, wnat[:, j], 1.0, wnat[:, j],
                            Alu.mult, Alu.mult, accum_out=acc)
                    else:
                        tr16 = tpool.tile([128, D], f16, tag="trash16",
                                          name=f"tra{t}_{j}")
                        nc.scalar.activation(tr16, wnat[:, j], Act.Square,
                                             accum_out=acc)
                    idx += 1
                joff += nj

            # rnorm = rsqrt(ssqg): quake seed + 2 Newton iterations (DVE)
            hT = spool.tile([128, tot_j], mybir.dt.int32, tag="hT",
                            name=f"hT{grp[0]}")
            iv = ssqg.bitcast(mybir.dt.int32)
            bshape = (128, tot_j)
            nc.vector.tensor_tensor(hT, iv, c_shift.to_broadcast(bshape),
                                    Alu.arith_shift_right)
            nc.vector.tensor_tensor(hT, hT, c_xor.to_broadcast(bshape),
                                    Alu.bitwise_xor)
            nc.vector.tensor_tensor(hT, hT, c_magic.to_broadcast(bshape),
                                    Alu.add)
            yv = hT.bitcast(f32)
            nt1 = spool.tile([128, tot_j], f32, tag="nt1",
                             name=f"nt1{grp[0]}")
            for _ in range(2):
                nc.vector.tensor_tensor(nt1, yv, yv, Alu.mult)
                nc.vector.tensor_tensor(nt1, nt1, ssqg, Alu.mult)
                nc.vector.tensor_scalar(nt1, nt1, -0.5, 1.5,
                                        Alu.mult, Alu.add)
                nc.vector.tensor_tensor(yv, yv, nt1, Alu.mult)

            joff = 0
            for gi, t in enumerate(grp):
                nj = njs[gi]
                cols = CT_SIZES[t]
                nh = cols // 512                    # 512-wide psum halves
                wnat = wnats[gi]
                for j in range(nj):
                    rn = yv[:, joff + j:joff + j + 1]
                    nc.vector.tensor_scalar(wnat[:, j], wnat[:, j], rn,
                                            None, Alu.mult)
                joff += nj

                # wT[p, j, k, c] = wn[j*128 + c, k*128 + p] (one merged
                # xbar transpose: in [128, nj*512] -> out [128, nj*4, 128])
                wT = wtpool.tile([128, nj, 4, 128], f16, tag="wT",
                                 name=f"wT{t}")
                nc.sync.dma_start_transpose(wT, wnat)

                for j2 in range(2):
                    pc = psum.tile([128, nh, 512], f32, tag="pc",
                                   name=f"pc{t}_{j2}")
                    for h in range(nh):
                        for k in range(4):
                            nc.tensor.matmul(
                                pc[:, h], lhsT=xnT[:, j2, k],
                                rhs=wT[:, 4 * h:4 * h + 4, k],
                                start=(k == 0), stop=(k == 3))
                    d2 = epool.tile([128, nh, 512], f32, tag="d2",
                                    name=f"d2_{t}_{j2}")
                    nc.scalar.activation(d2, pc, Act.Square,
                                         bias=nega[:, j2:j2 + 1])
                    f_ = epool.tile([128, nh, 512], f32, tag="f",
                                    name=f"f_{t}_{j2}")
                    nc.scalar.activation(f_, d2, Act.Exp, bias=lnk1,
                                         scale=-1.0 / SIGMA)
                    s_ = epool.tile([128, nh, 512], f32, tag="s",
                                    name=f"s_{t}_{j2}")
                    nc.vector.scalar_tensor_tensor(s_, pc, 1.0, f_,
                                                   Alu.add, Alu.mult)
                    o_ = opool.tile([128, nh, 512], f16, tag="o",
                                    name=f"o_{t}_{j2}")
                    nc.vector.tensor_scalar(o_, s_, SCALE, None,
                                            Alu.subtract)
                    nc.sync.dma_start(
                        out_e[j2 * 128:(j2 + 1) * 128,
                              ct_start[t]:ct_start[t] + cols], o_)

    nc.compile()
    return nc


def _get_nc():
    nc = _CACHE.get("nc")
    if nc is None:
        nc = _build()
        _CACHE["nc"] = nc
    return nc


def _run(in_maps, trace=False, tmpdir=None):
    from concourse.bass_utils import run_bass_kernel_spmd

    nc = _get_nc()
    return run_bass_kernel_spmd(
        nc, in_maps, core_ids=list(range(NCORES)), trace=trace, tmpdir=tmpdir)


def make_in_maps(input, label, weight):
    inp = np.ascontiguousarray(np.asarray(input, dtype=np.float32))
    lab = np.asarray(label).astype(np.int64)
    w = np.ascontiguousarray(np.asarray(weight, dtype=np.float32))
    wlab = np.ascontiguousarray(w[lab])
    wpad = np.concatenate([w, np.ones((CPAD - C, D), np.float32)], axis=0)
    in_maps = [
        {"inp": inp, "wlab": wlab,
         "w": np.ascontiguousarray(wpad[i * CSH:(i + 1) * CSH])}
        for i in range(NCORES)
    ]
    return in_maps, lab


def assemble(results, lab):
    full = np.concatenate(
        [results[i]["out"] for i in range(NCORES)], axis=1
    )[:, :C].astype(np.float32)
    alb = np.asarray(results[0]["alb"], dtype=np.float32)  # [128, 2]
    a_vec = alb.transpose(1, 0).reshape(B)
    full[np.arange(B), lab] = (SCALE * a_vec).astype(np.float32)
    return full


def kernel(input, label, weight):
    in_maps, lab = make_in_maps(input, label, weight)
    res = _run(in_maps)
    return assemble(res.results, lab)
